# revision 22
# baseline (speedup 1.0000x reference)
"""Trainium2 Bass kernel for nn_Attention_32401233281111 (attention pooling).

Reference computation (per batch b of 2048, N=64, D=512, H=16):
    h    = tanh(z @ W1 + b1)        # [B, N, H]
    w    = h @ W2                   # [B, N, 1]
    beta = softmax(w, axis=N)       # [B, N, 1]
    pooled = sum_n beta * z         # [B, D]
    returns (pooled, beta)

Sharding: data-parallel on batch across 8 NeuronCores (256 batches/core);
params replicated; no cross-core communication.

Per-core layout ("b-major"): two groups of 128 batches live on the 128 SBUF
partitions. For each n-slice z_n [128b, 512d]:
  phase A: PE transposes 4 128x128 chunks -> PSUM, copy to SBUF (DVE/ACT
           split), 4 accumulating matmuls vs W1 chunks -> h [128b, 16];
           per 4 slices: bias-add + tanh + W2 dot -> logits W_grp[:, n].
  softmax: free-dim max / exp(bias=-max, accum_out=S) / recip / scale.
  phase C: diag(beta_n) built as ident * beta_col (DVE 2x mode), then
           64 accumulating PE matmuls diag^T @ z_n into one PSUM bank
           (a tunable subset runs as DVE fused MACs instead, merged at
           the end) -> pooled[128, 512].
"""

import os
import sys

import numpy as np

_TRN_REPO = "/opt/trn_rl_repo"
if _TRN_REPO not in sys.path:
    sys.path.insert(0, _TRN_REPO)

import concourse.bass as bass  # noqa: E402
import concourse.bacc as bacc  # noqa: E402
import concourse.mybir as mybir  # noqa: E402
import concourse.tile as tile  # noqa: E402
from concourse.bass_utils import run_bass_kernel_spmd  # noqa: E402

F32 = mybir.dt.float32
ALU = mybir.AluOpType
ACTF = mybir.ActivationFunctionType

N_CORES = 8
B_FULL = 2048
N = 64
D = 512
HID = 16
P = 128
B_CORE = B_FULL // N_CORES  # 256
N_GROUPS = B_CORE // P  # 2
N_DCHUNK = D // P  # 4

# --- engine-balance tunables -------------------------------------------------
# zT PSUM->SBUF copy: n % COPY_DVE_MOD == 0 goes to DVE, else ACT (scalar).
COPY_DVE_MOD = 1000000
# phase-C MAC residues (mod 16) assigned to GPSIMD (2-op MACs there)
MAC_POOL_RES = ()
# phase C: n % PHC_DVE_MOD == PHC_DVE_MOD - 1 runs as a DVE fused MAC chain
# instead of a PE diag-matmul.
PHC_DVE_MOD = 6
# z-slice pool depth: slices free right after their phase-C MAC, so only
# a modest lookahead window is needed.
Z_BUFS = 12
# n-slices loaded per DMA (amortizes HWDGE per-transfer overhead)
LOAD_Q = 4


def build_kernel(n_groups: int = N_GROUPS, n_slices: int = N, reps: int = 1):
    """Build the per-core Bass program. Returns nc.

    reps > 1 repeats the whole computation (for benchmarking: amortizes
    launch overhead; outputs are simply overwritten each rep).
    """
    nc = bacc.Bacc("TRN2", target_bir_lowering=False, debug=False)

    b_core = n_groups * P
    zs = nc.dram_tensor("zs", [b_core, n_slices, D], F32, kind="ExternalInput").ap()
    # W1 rearranged on host to [128, 4*16]: w1t[p, c*16+h] = W1[c*128+p, h]
    w1t = nc.dram_tensor("w1t", [P, N_DCHUNK * HID], F32, kind="ExternalInput").ap()
    # b1 / W2 broadcast to all partitions, tiled x4 on host: [128, 64]
    b1b = nc.dram_tensor("b1b", [P, 4 * HID], F32, kind="ExternalInput").ap()
    w2b = nc.dram_tensor("w2b", [P, 4 * HID], F32, kind="ExternalInput").ap()
    ident = nc.dram_tensor("ident", [P, P], F32, kind="ExternalInput").ap()

    pooled_o = nc.dram_tensor("pooled_o", [b_core, D], F32, kind="ExternalOutput").ap()
    beta_o = nc.dram_tensor("beta_o", [b_core, n_slices], F32, kind="ExternalOutput").ap()

    with tile.TileContext(nc, pool_alloc_mode="queue") as tc:
        with (
            tc.tile_pool(name="const", bufs=1) as const,
            tc.tile_pool(name="zsl", bufs=Z_BUFS) as zsl,
            tc.tile_pool(name="ztp", bufs=6) as ztp,
            tc.tile_pool(name="work", bufs=8) as work,
            tc.tile_pool(name="wgrp", bufs=2) as wgrp,
            tc.tile_pool(name="accp", bufs=2) as accp,
            tc.tile_pool(name="outp", bufs=2) as outp,
            tc.tile_pool(name="pst", bufs=3, space="PSUM") as pst,
            tc.tile_pool(name="hps", bufs=2, space="PSUM") as hps,
        ):
            # constants
            w1_sb = const.tile([P, N_DCHUNK * HID], F32)
            nc.sync.dma_start(out=w1_sb, in_=w1t)
            b1_sb = const.tile([P, 4 * HID], F32)
            nc.sync.dma_start(out=b1_sb, in_=b1b)
            w2_sb = const.tile([P, 4 * HID], F32)
            nc.sync.dma_start(out=w2_sb, in_=w2b)
            id_sb = const.tile([P, P], F32)
            nc.sync.dma_start(out=id_sb, in_=ident)

            for g in range(n_groups * reps):
                g = g % n_groups
                b0 = g * P
                # unnormalized softmax: logits are bounded (|w| <= sum|W2|
                # <= 4), so exp(w) is safe in fp32 without max-subtraction.
                # e_grp[:, n] = exp(w_n) is produced per quad of slices, and
                # the phase-C MAC for slice n runs immediately after -- no
                # group-wide softmax barrier, no tail.
                assert n_slices % 4 == 0
                e_grp = wgrp.tile([P, n_slices], F32, tag="egrp")
                zg = []
                state = {"acc_v": None, "acc_p": None, "h4": None}
                n_pairs = n_slices // 2
                pair_ps = {}
                pair_zq = {}

                def emit_pair_front(p):
                    # loads (one DMA per LOAD_Q slices) + 8 transposes
                    for n in (2 * p, 2 * p + 1):
                        if n % LOAD_Q == 0:
                            zq = zsl.tile([P, LOAD_Q * D], F32, tag="z")
                            nc.sync.dma_start(
                                out=zq,
                                in_=zs[b0 : b0 + P, n : n + LOAD_Q, :].rearrange(
                                    "p n d -> p (n d)"
                                ),
                            )
                            pair_zq[n // LOAD_Q] = zq
                        zq = pair_zq[n // LOAD_Q]
                        zg.append(zq[:, (n % LOAD_Q) * D : (n % LOAD_Q + 1) * D])
                    ps2 = pst.tile([P, 2 * D], F32, tag="pst")
                    pair_ps[p] = ps2
                    for h in range(2):
                        z_n = zg[2 * p + h]
                        for j in range(N_DCHUNK):
                            nc.tensor.transpose(
                                ps2[:, h * D + j * P : h * D + (j + 1) * P],
                                z_n[:, j * P : (j + 1) * P],
                                id_sb,
                            )

                def emit_pair_back(p):
                    ps2 = pair_ps.pop(p)
                    zt2 = ztp.tile([P, 2 * D], F32, tag="zt")
                    if p % COPY_DVE_MOD == 0:
                        nc.vector.tensor_copy(zt2, ps2)
                    else:
                        nc.scalar.copy(zt2, ps2)
                    for h in range(2):
                        nn = 2 * p + h
                        k = nn % 4
                        if k == 0:
                            state["h4"] = hps.tile([P, 4 * HID], F32, tag="h4", name="h4t")
                        h4 = state["h4"]
                        for j in range(N_DCHUNK):
                            nc.tensor.matmul(
                                h4[:, k * HID : (k + 1) * HID],
                                lhsT=zt2[:, h * D + j * P : h * D + (j + 1) * P],
                                rhs=w1_sb[:, j * HID : (j + 1) * HID],
                                start=(j == 0),
                                stop=(j == N_DCHUNK - 1),
                            )
                    if p % 2 == 0:
                        return
                    # quad complete: logits, exp, and phase-C MACs
                    n = 2 * p + 1
                    q0 = n - 3
                    h4 = state["h4"]
                    hb = work.tile([P, 4 * HID], F32, tag="hb")
                    nc.vector.tensor_add(hb, h4, b1_sb)
                    ht = work.tile([P, 4 * HID], F32, tag="ht")
                    nc.scalar.activation(ht, hb, ACTF.Tanh)
                    pr = work.tile([P, 4 * HID], F32, tag="pr")
                    nc.gpsimd.tensor_mul(pr, ht, w2_sb)
                    w4 = work.tile([P, 4], F32, tag="w4")
                    nc.vector.tensor_reduce(
                        w4,
                        pr.rearrange("p (s h) -> p s h", h=HID),
                        axis=mybir.AxisListType.X,
                        op=ALU.add,
                    )
                    nc.scalar.activation(e_grp[:, q0 : n + 1], w4, ACTF.Exp)
                    for m in range(q0, n + 1):
                        ecol = e_grp[:, m : m + 1]
                        # GPSIMD has no fused scalar_tensor_tensor (walrus ISA
                        # check rejects it on Pool): its MACs are 2 ops.
                        if m % 16 not in MAC_POOL_RES:
                            if state["acc_v"] is None:
                                state["acc_v"] = accp.tile([P, D], F32, tag="accv", name="accv")
                                nc.vector.tensor_scalar(
                                    state["acc_v"], zg[m], ecol, None, op0=ALU.mult
                                )
                            else:
                                nc.vector.scalar_tensor_tensor(
                                    state["acc_v"], zg[m], ecol, state["acc_v"],
                                    op0=ALU.mult, op1=ALU.add,
                                )
                        else:
                            if state["acc_p"] is None:
                                state["acc_p"] = accp.tile([P, D], F32, tag="accp", name="accp")
                                nc.gpsimd.tensor_scalar(
                                    state["acc_p"], zg[m], ecol, None, op0=ALU.mult
                                )
                            else:
                                tmp = accp.tile([P, D], F32, tag="ptmp", name="ptmp")
                                nc.gpsimd.tensor_scalar(
                                    tmp, zg[m], ecol, None, op0=ALU.mult
                                )
                                nc.gpsimd.tensor_add(
                                    state["acc_p"], state["acc_p"], tmp
                                )

                # software-pipelined emission: pair p's front (loads +
                # transposes) goes out before pair p-1's back half, so PE
                # never head-of-line blocks on the PSUM->SBUF copy.
                for p in range(n_pairs):
                    emit_pair_front(p)
                    if p > 0:
                        emit_pair_back(p - 1)
                emit_pair_back(n_pairs - 1)
                acc_v = state["acc_v"]
                acc_p = state["acc_p"]

                # normalization: S = sum_n exp(w_n); beta = e/S; pooled/S
                ssum = work.tile([P, 1], F32, tag="ssum")
                nc.vector.tensor_reduce(
                    ssum, e_grp, axis=mybir.AxisListType.X, op=ALU.add
                )
                sinv = work.tile([P, 1], F32, tag="sinv")
                nc.vector.reciprocal(sinv, ssum)
                beta_t = wgrp.tile([P, n_slices], F32, tag="beta")
                nc.vector.tensor_scalar_mul(beta_t, e_grp, sinv)
                nc.scalar.dma_start(out=beta_o[b0 : b0 + P, :], in_=beta_t)

                po = outp.tile([P, D], F32, tag="po")
                if acc_v is not None and acc_p is not None:
                    nc.vector.tensor_add(po, acc_v, acc_p)
                elif acc_v is not None:
                    nc.vector.tensor_copy(po, acc_v)
                else:
                    nc.vector.tensor_copy(po, acc_p)
                nc.vector.tensor_scalar_mul(po, po, sinv)
                nc.scalar.dma_start(out=pooled_o[b0 : b0 + P, :], in_=po)

    nc.compile()
    return nc


def _prep_const_inputs(W1: np.ndarray, b1: np.ndarray, W2: np.ndarray):
    w1t = (
        np.ascontiguousarray(
            W1.reshape(N_DCHUNK, P, HID).transpose(1, 0, 2).reshape(P, N_DCHUNK * HID)
        )
        .astype(np.float32)
    )
    b1b = np.tile(b1.reshape(1, HID), (P, 4)).astype(np.float32)
    w2b = np.tile(W2.reshape(1, HID), (P, 4)).astype(np.float32)
    ident = np.eye(P, dtype=np.float32)
    return w1t, b1b, w2b, ident


_NC_CACHE = {}


def kernel(z, W1, b1, W2, _trace=False):
    z = np.ascontiguousarray(np.asarray(z, dtype=np.float32))
    W1 = np.asarray(W1, dtype=np.float32)
    b1 = np.asarray(b1, dtype=np.float32)
    W2 = np.asarray(W2, dtype=np.float32)
    assert z.shape == (B_FULL, N, D), z.shape

    key = "full"
    if key not in _NC_CACHE:
        _NC_CACHE[key] = build_kernel()
    nc = _NC_CACHE[key]

    w1t, b1b, w2b, ident = _prep_const_inputs(W1, b1, W2)
    in_maps = []
    for c in range(N_CORES):
        in_maps.append(
            {
                "zs": z[c * B_CORE : (c + 1) * B_CORE],
                "w1t": w1t,
                "b1b": b1b,
                "w2b": w2b,
                "ident": ident,
            }
        )

    res = run_bass_kernel_spmd(
        nc, in_maps, core_ids=list(range(N_CORES)), trace=_trace
    )

    pooled = np.concatenate([r["pooled_o"] for r in res.results], axis=0)
    beta = np.concatenate([r["beta_o"] for r in res.results], axis=0)
    beta = beta.reshape(B_FULL, N, 1)

    if _trace:
        kernel.last_results = res
    return pooled.astype(np.float32), beta.astype(np.float32)


if __name__ == "__main__":
    # smoke test with random data
    rng = np.random.default_rng(0)
    z = rng.standard_normal((B_FULL, N, D), dtype=np.float32)
    W1 = rng.standard_normal((D, HID), dtype=np.float32) * 0.04
    b1 = rng.standard_normal(HID).astype(np.float32) * 0.04
    W2 = rng.standard_normal((HID, 1), dtype=np.float32) * 0.25
    pooled, beta = kernel(z, W1, b1, W2)
    print("pooled", pooled.shape, "beta", beta.shape)
